# revision 1
# baseline (speedup 1.0000x reference)
"""DCCA 2D loss kernel for 8 Trainium2 NeuronCores (Bass/Tile).

Strategy (data-parallel over the m = B*C = 2048 sample axis):
  - Each core gets 256 samples of both views. Per 4-sample "quad" it loads
    both views with 128-partition DMAs (partition = (sample-in-quad)*32 +
    n//2, so every DMA descriptor reads a contiguous 1KB row-pair), PE-
    transposes the four [128,128] slices, assembles per-sample fused tiles
    T_m = [H1_m^T | H2_m^T] (the n-axis lands in a fixed even/odd
    permutation, under which the final scalar is exactly invariant), and
    accumulates the fused Gram  G += T_m^T T_m  in PSUM.  G's 64x64 blocks
    are [[S11raw, S12raw], [S21raw, S22raw]].
  - AllGather of the per-core partial G (64KB), summed on every core.
  - Replicated epilogue without eigh:  with  S = c1*G + R*I  and
    A = blockdiag(S11, S22),   corr^2 = trace(S11^-1 S12 S22^-1 S12^T)
    = sum( (A^-1 S)[0:64,64:128] * (S A^-1)[0:64,64:128] ),
    where A^-1 comes from Newton-Schulz iterations (X <- X(2I - AX),
    X0 = (n/tr A) I; condition number ~1.02 so 4 iterations are exact to
    fp32).  Output = -sqrt(corr^2).
"""

import os

import numpy as np

# ---------------------------------------------------------------- constants
B, C, N, K = 32, 64, 64, 128
M = B * C                    # 2048 samples
NC = 8                       # cores
NS = M // NC                 # 256 samples per core
CH = 64                      # samples per DMA chunk (per view)
QC = CH // 4                 # quads per chunk
NCHUNK = NS // CH
R_RIDGE = 1e-4
C1 = float((1.0 - 1.0 / M) ** 2 / (M * (M - 1)))  # Gram -> Sigma scale

# "bf16" (fast, ~1e-4 rel err), "f32r" (reduced fp32 matmul), "f32" (safest)
GRAM_MODE = os.environ.get("BASS_GRAM_MODE", "bf16")
NEWTON_ITERS = int(os.environ.get("BASS_NEWTON_ITERS", "4"))

_CACHE = {}


def _gdt(mybir):
    return {
        "bf16": mybir.dt.bfloat16,
        "f32r": mybir.dt.float32r,
        "f32": mybir.dt.float32,
    }[GRAM_MODE]


def _gnp():
    if GRAM_MODE == "bf16":
        import ml_dtypes

        return ml_dtypes.bfloat16
    return np.float32


def _build():
    import concourse.bass as bass
    import concourse.mybir as mybir
    import concourse.tile as tile
    from concourse import bacc

    gdt = _gdt(mybir)
    f32 = mybir.dt.float32

    nc = bacc.Bacc(
        "TRN2",
        target_bir_lowering=False,
        debug=False,
        enable_asserts=False,
        num_devices=NC,
    )

    x1 = nc.dram_tensor("x1", [NS, N, K], f32, kind="ExternalInput").ap()
    x2 = nc.dram_tensor("x2", [NS, N, K], f32, kind="ExternalInput").ap()
    ident_d = nc.dram_tensor("ident", [128, 128], gdt, kind="ExternalInput").ap()
    eye2_d = nc.dram_tensor("eye2", [128, 128], f32, kind="ExternalInput").ap()
    reye_d = nc.dram_tensor("reye", [128, 128], f32, kind="ExternalInput").ap()
    maskd_d = nc.dram_tensor("maskd", [128, 128], f32, kind="ExternalInput").ap()
    ones_d = nc.dram_tensor("onesf", [128, 128], f32, kind="ExternalInput").ap()
    out_d = nc.dram_tensor("out", [1, 1], f32, kind="ExternalOutput").ap()

    with tile.TileContext(nc) as tc:
        import contextlib

        with contextlib.ExitStack() as ctx:
            cpool = ctx.enter_context(tc.tile_pool(name="consts", bufs=1))
            spool = ctx.enter_context(tc.tile_pool(name="work", bufs=2))

            ident = cpool.tile([128, 128], gdt)
            nc.sync.dma_start(ident[:], ident_d)
            eye2 = cpool.tile([128, 128], f32)
            nc.sync.dma_start(eye2[:], eye2_d)
            reye = cpool.tile([128, 128], f32)
            nc.sync.dma_start(reye[:], reye_d)
            maskd = cpool.tile([128, 128], f32)
            nc.sync.dma_start(maskd[:], maskd_d)
            onesf = cpool.tile([128, 128], f32)
            nc.sync.dma_start(onesf[:], ones_d)

            gsb = spool.tile([128, 128], f32, tag="gsb")

            # ---------------- main loop: per-core partial fused Gram ------
            with (
                tc.tile_pool(name="vload", bufs=2) as vpool,
                tc.tile_pool(name="ttp", bufs=4) as ttpool,
                tc.tile_pool(name="ptp", bufs=4, space="PSUM") as ptpool,
                tc.tile_pool(name="gpp", bufs=1, space="PSUM") as gpool,
            ):
                if GRAM_MODE == "f32r":
                    gpA = gpool.tile([128, 256], f32, tag="gpA")
                    gpB = gpool.tile([128, 256], f32, tag="gpB")
                else:
                    gp = gpool.tile([128, 128], f32, tag="gp")

                first = [True, True]  # start-flags for (gpA, gpB) / (gp,)
                for ci in range(NCHUNK):
                    vts = []
                    for vi, xsrc in enumerate((x1, x2)):
                        vt = vpool.tile([128, QC, 256], gdt, tag=f"v{vi}")
                        src = xsrc[ci * CH : (ci + 1) * CH].rearrange(
                            "(q h) (u r) k -> (h u) q (r k)", h=4, r=2
                        )
                        if gdt == f32:
                            nc.sync.dma_start(vt[:], src)
                        else:
                            # SWDGE handles the dtype cast (bf16) and the
                            # f32->f32r relabel during the transfer.
                            nc.gpsimd.dma_start(vt[:], src)
                        vts.append(vt)

                    for q in range(QC):
                        tt4 = ttpool.tile([128, 512], gdt, tag="tt4")
                        tt4_blk = tt4.rearrange("p (h b u) -> p h b u", h=4, u=32)
                        for vi in range(2):
                            for r in range(2):
                                pt = ptpool.tile([128, 128], f32, tag="pt")
                                nc.tensor.transpose(
                                    pt[:], vts[vi][:, q, r * 128 : (r + 1) * 128], ident[:]
                                )
                                nc.any.tensor_copy(
                                    out=tt4_blk[:, :, 2 * vi + r, :],
                                    in_=pt.rearrange("p (h u) -> p h u", u=32),
                                )
                        last = ci == NCHUNK - 1 and q == QC - 1
                        if GRAM_MODE == "f32r":
                            for h in range(4):
                                acc = gpA if h % 2 == 0 else gpB
                                fi = 0 if h % 2 == 0 else 1
                                nc.tensor.matmul(
                                    acc[:],
                                    tt4[:, 128 * h : 128 * (h + 1)],
                                    tt4[:, 256 * (h // 2) : 256 * (h // 2 + 1)],
                                    start=first[fi],
                                    stop=last and h >= 2,
                                )
                                first[fi] = False
                        else:
                            for h in range(4):
                                nc.tensor.matmul(
                                    gp[:],
                                    tt4[:, 128 * h : 128 * (h + 1)],
                                    tt4[:, 128 * h : 128 * (h + 1)],
                                    start=first[0],
                                    stop=last and h == 3,
                                )
                                first[0] = False

                if GRAM_MODE == "f32r":
                    nc.vector.tensor_add(gsb[:], gpA[:, 0:128], gpB[:, 128:256])
                else:
                    nc.vector.tensor_copy(gsb[:], gp[:])

            # ---------------- AllGather + replicated epilogue -------------
            with (
                tc.tile_pool(name="dram", bufs=1, space="DRAM") as dpool,
                tc.tile_pool(name="epp", bufs=1, space="PSUM") as epool,
            ):
                din = dpool.tile([128, 128], f32)
                dout = dpool.tile([NC, 128, 128], f32)
                nc.gpsimd.dma_start(din[:], gsb[:])
                nc.gpsimd.collective_compute(
                    "AllGather",
                    mybir.AluOpType.bypass,
                    replica_groups=[list(range(NC))],
                    ins=[din.opt()],
                    outs=[dout.opt()],
                )
                gall = spool.tile([128, NC, 128], f32, tag="gall")
                nc.sync.dma_start(gall[:], dout[:].rearrange("c p k -> p c k"))

                g4 = spool.tile([128, 4, 128], f32, tag="g4")
                nc.vector.tensor_add(g4[:], gall[:, 0:4, :], gall[:, 4:8, :])
                g2 = spool.tile([128, 2, 128], f32, tag="g2")
                nc.vector.tensor_add(g2[:], g4[:, 0:2, :], g4[:, 2:4, :])
                gs = spool.tile([128, 128], f32, tag="gs")
                nc.vector.tensor_add(gs[:], g2[:, 0, :], g2[:, 1, :])

                # S = c1*G + R*I ; A = blockdiag(S)
                S = spool.tile([128, 128], f32, tag="S")
                nc.vector.tensor_scalar_mul(S[:], gs[:], C1)
                nc.vector.tensor_add(S[:], S[:], reye[:])
                A = spool.tile([128, 128], f32, tag="A")
                nc.vector.tensor_mul(A[:], S[:], maskd[:])

                # X0 = (128 / tr(A)) * I   via (A*2I) row-sums -> ones bcast
                dm = spool.tile([128, 128], f32, tag="dm")
                nc.vector.tensor_mul(dm[:], A[:], eye2[:])
                dcol = spool.tile([128, 1], f32, tag="dcol")
                nc.vector.reduce_sum(dcol[:], dm[:], axis=mybir.AxisListType.X)
                trp = epool.tile([128, 1], f32, tag="trp")
                nc.tensor.matmul(trp[:], onesf[:], dcol[:], start=True, stop=True)
                trs = spool.tile([128, 1], f32, tag="trs")
                nc.vector.tensor_copy(trs[:], trp[:])
                rcol = spool.tile([128, 1], f32, tag="rcol")
                nc.vector.reciprocal(rcol[:], trs[:])
                xcur = spool.tile([128, 128], f32, tag="xn")
                nc.vector.tensor_scalar(
                    xcur[:], eye2[:], rcol[:], 128.0,
                    op0=mybir.AluOpType.mult, op1=mybir.AluOpType.mult,
                )

                for _ in range(NEWTON_ITERS):
                    bp = epool.tile([128, 128], f32, tag="bp")
                    nc.tensor.matmul(bp[:], A[:], xcur[:], start=True, stop=True)
                    cs = spool.tile([128, 128], f32, tag="cs")
                    nc.vector.tensor_tensor(
                        cs[:], eye2[:], bp[:], mybir.AluOpType.subtract
                    )
                    xp = epool.tile([128, 128], f32, tag="xp")
                    nc.tensor.matmul(xp[:], xcur[:], cs[:], start=True, stop=True)
                    xnew = spool.tile([128, 128], f32, tag="xn")
                    nc.vector.tensor_copy(xnew[:], xp[:])
                    xcur = xnew

                up = epool.tile([128, 128], f32, tag="up")
                nc.tensor.matmul(up[:], xcur[:], S[:], start=True, stop=True)
                vp = epool.tile([128, 128], f32, tag="vp")
                nc.tensor.matmul(vp[:], S[:], xcur[:], start=True, stop=True)
                us = spool.tile([64, 64], f32, tag="us")
                nc.vector.tensor_copy(us[:], up[0:64, 64:128])
                pm = spool.tile([64, 64], f32, tag="pm")
                nc.vector.tensor_tensor(
                    pm[:], us[:], vp[0:64, 64:128], mybir.AluOpType.mult
                )
                pcol = spool.tile([64, 1], f32, tag="pcol")
                nc.vector.reduce_sum(pcol[:], pm[:], axis=mybir.AxisListType.X)
                cp = epool.tile([1, 1], f32, tag="cp")
                nc.tensor.matmul(cp[:], pcol[:], onesf[0:64, 0:1], start=True, stop=True)
                c2 = spool.tile([1, 1], f32, tag="c2")
                nc.vector.tensor_copy(c2[:], cp[:])
                root = spool.tile([1, 1], f32, tag="root")
                nc.scalar.sqrt(root[:], c2[:])
                nc.vector.tensor_scalar_mul(root[:], root[:], -1.0)
                nc.sync.dma_start(out_d, root[:])

    nc.compile()
    return nc


def _get_nc():
    key = (GRAM_MODE, NEWTON_ITERS)
    if key not in _CACHE:
        _CACHE[key] = _build()
    return _CACHE[key]


def _const_inputs():
    eye = np.eye(128, dtype=np.float32)
    maskd = np.zeros((128, 128), dtype=np.float32)
    maskd[:64, :64] = np.eye(64, dtype=np.float32)
    maskd[64:, 64:] = np.eye(64, dtype=np.float32)
    return {
        "ident": np.eye(128).astype(_gnp()),
        "eye2": (2.0 * eye).astype(np.float32),
        "reye": (R_RIDGE * eye).astype(np.float32),
        "maskd": maskd,
        "onesf": np.ones((128, 128), dtype=np.float32),
    }


def kernel(data_view1, data_view2):
    from concourse import bass_utils

    h1 = np.ascontiguousarray(data_view1, dtype=np.float32).reshape(M, N, K)
    h2 = np.ascontiguousarray(data_view2, dtype=np.float32).reshape(M, N, K)

    consts = _const_inputs()
    in_maps = []
    for c in range(NC):
        m = {
            "x1": h1[c * NS : (c + 1) * NS],
            "x2": h2[c * NS : (c + 1) * NS],
        }
        m.update(consts)
        in_maps.append(m)

    nc = _get_nc()
    trace = os.environ.get("BASS_KERNEL_TRACE", "0") == "1"
    res = bass_utils.run_bass_kernel_spmd(
        nc, in_maps, core_ids=list(range(NC)), trace=trace
    )
    if trace:
        kernel.last_results = res
    val = np.asarray(res.results[0]["out"]).reshape(())
    return val.astype(np.float32)


# revision 2
# speedup vs baseline: 1.3559x; 1.3559x over previous
"""DCCA 2D loss kernel for 8 Trainium2 NeuronCores (Bass/Tile).

Strategy (data-parallel over the m = B*C = 2048 sample axis):
  - Each core gets 256 samples of both views. Per 4-sample "quad" it loads
    both views with 128-partition DMAs (partition = (sample-in-quad)*32 +
    n//2, so every DMA descriptor reads a contiguous 1KB row-pair), PE-
    transposes the four [128,128] slices, assembles per-sample fused tiles
    T_m = [H1_m^T | H2_m^T] (the n-axis lands in a fixed even/odd
    permutation, under which the final scalar is exactly invariant), and
    accumulates the fused Gram  G += T_m^T T_m  in PSUM.  G's 64x64 blocks
    are [[S11raw, S12raw], [S21raw, S22raw]].
  - AllGather of the per-core partial G (64KB), summed on every core.
  - Replicated epilogue without eigh:  with  S = c1*G + R*I  and
    A = blockdiag(S11, S22),   corr^2 = trace(S11^-1 S12 S22^-1 S12^T)
    = sum( (A^-1 S)[0:64,64:128] * (S A^-1)[0:64,64:128] ),
    where A^-1 comes from Newton-Schulz iterations (X <- X(2I - AX),
    X0 = (n/tr A) I; condition number ~1.02 so 4 iterations are exact to
    fp32).  Output = -sqrt(corr^2).
"""

import os

import numpy as np

# ---------------------------------------------------------------- constants
B, C, N, K = 32, 64, 64, 128
M = B * C                    # 2048 samples
NC = 8                       # cores
NS = M // NC                 # 256 samples per core
CH = 64                      # samples per DMA chunk (per view)
QC = CH // 4                 # quads per chunk
NCHUNK = NS // CH
R_RIDGE = 1e-4
C1 = float((1.0 - 1.0 / M) ** 2 / (M * (M - 1)))  # Gram -> Sigma scale

# "bf16" (fast, ~1e-4 rel err), "f32r" (reduced fp32 matmul), "f32" (safest)
GRAM_MODE = os.environ.get("BASS_GRAM_MODE", "bf16")
NEWTON_ITERS = int(os.environ.get("BASS_NEWTON_ITERS", "4"))

_CACHE = {}


def _gdt(mybir):
    return {
        "bf16": mybir.dt.bfloat16,
        "f32r": mybir.dt.float32r,
        "f32": mybir.dt.float32,
    }[GRAM_MODE]


def _gnp():
    if GRAM_MODE == "bf16":
        import ml_dtypes

        return ml_dtypes.bfloat16
    return np.float32


def _build():
    import concourse.bass as bass
    import concourse.mybir as mybir
    import concourse.tile as tile
    from concourse import bacc

    gdt = _gdt(mybir)
    f32 = mybir.dt.float32

    nc = bacc.Bacc(
        "TRN2",
        target_bir_lowering=False,
        debug=False,
        enable_asserts=False,
        num_devices=NC,
    )

    x1 = nc.dram_tensor("x1", [NS, N, K], f32, kind="ExternalInput").ap()
    x2 = nc.dram_tensor("x2", [NS, N, K], f32, kind="ExternalInput").ap()
    ident_d = nc.dram_tensor("ident", [128, 128], gdt, kind="ExternalInput").ap()
    eye2_d = nc.dram_tensor("eye2", [128, 128], f32, kind="ExternalInput").ap()
    reye_d = nc.dram_tensor("reye", [128, 128], f32, kind="ExternalInput").ap()
    maskd_d = nc.dram_tensor("maskd", [128, 128], f32, kind="ExternalInput").ap()
    ones_d = nc.dram_tensor("onesf", [128, 128], f32, kind="ExternalInput").ap()
    out_d = nc.dram_tensor("out", [1, 1], f32, kind="ExternalOutput").ap()

    with tile.TileContext(nc) as tc:
        import contextlib

        with contextlib.ExitStack() as ctx:
            cpool = ctx.enter_context(tc.tile_pool(name="consts", bufs=1))
            spool = ctx.enter_context(tc.tile_pool(name="work", bufs=2))

            ident = cpool.tile([128, 128], gdt)
            nc.sync.dma_start(ident[:], ident_d)
            eye2 = cpool.tile([128, 128], f32)
            nc.sync.dma_start(eye2[:], eye2_d)
            reye = cpool.tile([128, 128], f32)
            nc.sync.dma_start(reye[:], reye_d)
            maskd = cpool.tile([128, 128], f32)
            nc.sync.dma_start(maskd[:], maskd_d)
            onesf = cpool.tile([128, 128], f32)
            nc.sync.dma_start(onesf[:], ones_d)

            gsb = spool.tile([128, 128], f32, tag="gsb")

            # ---------------- main loop: per-core partial fused Gram ------
            with (
                tc.tile_pool(name="vload", bufs=2) as vpool,
                tc.tile_pool(name="ttp", bufs=4) as ttpool,
                tc.tile_pool(name="ptp", bufs=4, space="PSUM") as ptpool,
                tc.tile_pool(name="gpp", bufs=1, space="PSUM") as gpool,
            ):
                if GRAM_MODE == "f32r":
                    gpA = gpool.tile([128, 256], f32, tag="gpA")
                    gpB = gpool.tile([128, 256], f32, tag="gpB")
                else:
                    gp = gpool.tile([128, 128], f32, tag="gp")

                first = [True, True]  # start-flags for (gpA, gpB) / (gp,)
                for ci in range(NCHUNK):
                    vts = []
                    for vi, xsrc in enumerate((x1, x2)):
                        vt = vpool.tile([128, QC, 256], gdt, tag=f"v{vi}")
                        src = xsrc[ci * CH : (ci + 1) * CH].rearrange(
                            "(q h) (u r) k -> (h u) q (r k)", h=4, r=2
                        )
                        if gdt == f32:
                            nc.sync.dma_start(vt[:], src)
                        else:
                            # SWDGE handles the dtype cast (bf16) and the
                            # f32->f32r relabel during the transfer.
                            nc.gpsimd.dma_start(vt[:], src)
                        vts.append(vt)

                    for q in range(QC):
                        tt4 = ttpool.tile([128, 512], gdt, tag="tt4")
                        tt4_blk = tt4.rearrange("p (h b u) -> p h b u", h=4, u=32)
                        for vi in range(2):
                            for r in range(2):
                                pt = ptpool.tile([128, 128], gdt, tag="pt")
                                nc.tensor.transpose(
                                    pt[:], vts[vi][:, q, r * 128 : (r + 1) * 128], ident[:]
                                )
                                nc.any.tensor_copy(
                                    out=tt4_blk[:, :, 2 * vi + r, :],
                                    in_=pt.rearrange("p (h u) -> p h u", u=32),
                                )
                        last = ci == NCHUNK - 1 and q == QC - 1
                        if GRAM_MODE == "f32r":
                            for h in range(4):
                                acc = gpA if h % 2 == 0 else gpB
                                fi = 0 if h % 2 == 0 else 1
                                nc.tensor.matmul(
                                    acc[:],
                                    tt4[:, 128 * h : 128 * (h + 1)],
                                    tt4[:, 256 * (h // 2) : 256 * (h // 2 + 1)],
                                    start=first[fi],
                                    stop=last and h >= 2,
                                )
                                first[fi] = False
                        else:
                            for h in range(4):
                                nc.tensor.matmul(
                                    gp[:],
                                    tt4[:, 128 * h : 128 * (h + 1)],
                                    tt4[:, 128 * h : 128 * (h + 1)],
                                    start=first[0],
                                    stop=last and h == 3,
                                )
                                first[0] = False

                if GRAM_MODE == "f32r":
                    nc.vector.tensor_add(gsb[:], gpA[:, 0:128], gpB[:, 128:256])
                else:
                    nc.vector.tensor_copy(gsb[:], gp[:])

            # ---------------- AllGather + replicated epilogue -------------
            with (
                tc.tile_pool(name="dram", bufs=1, space="DRAM") as dpool,
                tc.tile_pool(name="epp", bufs=1, space="PSUM") as epool,
            ):
                din = dpool.tile([128, 128], f32)
                dout = dpool.tile([NC, 128, 128], f32)
                nc.gpsimd.dma_start(din[:], gsb[:])
                nc.gpsimd.collective_compute(
                    "AllGather",
                    mybir.AluOpType.bypass,
                    replica_groups=[list(range(NC))],
                    ins=[din.opt()],
                    outs=[dout.opt()],
                )
                gall = spool.tile([128, NC, 128], f32, tag="gall")
                nc.sync.dma_start(gall[:], dout[:].rearrange("c p k -> p c k"))

                g4 = spool.tile([128, 4, 128], f32, tag="g4")
                nc.vector.tensor_add(g4[:], gall[:, 0:4, :], gall[:, 4:8, :])
                g2 = spool.tile([128, 2, 128], f32, tag="g2")
                nc.vector.tensor_add(g2[:], g4[:, 0:2, :], g4[:, 2:4, :])
                gs = spool.tile([128, 128], f32, tag="gs")
                nc.vector.tensor_add(gs[:], g2[:, 0, :], g2[:, 1, :])

                # S = c1*G + R*I ; A = blockdiag(S)
                S = spool.tile([128, 128], f32, tag="S")
                nc.vector.tensor_scalar_mul(S[:], gs[:], C1)
                nc.vector.tensor_add(S[:], S[:], reye[:])
                A = spool.tile([128, 128], f32, tag="A")
                nc.vector.tensor_mul(A[:], S[:], maskd[:])

                # X0 = (128 / tr(A)) * I   via (A*2I) row-sums -> ones bcast
                dm = spool.tile([128, 128], f32, tag="dm")
                nc.vector.tensor_mul(dm[:], A[:], eye2[:])
                dcol = spool.tile([128, 1], f32, tag="dcol")
                nc.vector.reduce_sum(dcol[:], dm[:], axis=mybir.AxisListType.X)
                trp = epool.tile([128, 1], f32, tag="trp")
                nc.tensor.matmul(trp[:], onesf[:], dcol[:], start=True, stop=True)
                trs = spool.tile([128, 1], f32, tag="trs")
                nc.vector.tensor_copy(trs[:], trp[:])
                rcol = spool.tile([128, 1], f32, tag="rcol")
                nc.vector.reciprocal(rcol[:], trs[:])
                xcur = spool.tile([128, 128], f32, tag="xn")
                nc.vector.tensor_scalar(
                    xcur[:], eye2[:], rcol[:], 128.0,
                    op0=mybir.AluOpType.mult, op1=mybir.AluOpType.mult,
                )

                for _ in range(NEWTON_ITERS):
                    bp = epool.tile([128, 128], f32, tag="bp")
                    nc.tensor.matmul(bp[:], A[:], xcur[:], start=True, stop=True)
                    cs = spool.tile([128, 128], f32, tag="cs")
                    nc.vector.tensor_tensor(
                        cs[:], eye2[:], bp[:], mybir.AluOpType.subtract
                    )
                    xp = epool.tile([128, 128], f32, tag="xp")
                    nc.tensor.matmul(xp[:], xcur[:], cs[:], start=True, stop=True)
                    xnew = spool.tile([128, 128], f32, tag="xn")
                    nc.vector.tensor_copy(xnew[:], xp[:])
                    xcur = xnew

                up = epool.tile([128, 128], f32, tag="up")
                nc.tensor.matmul(up[:], xcur[:], S[:], start=True, stop=True)
                vp = epool.tile([128, 128], f32, tag="vp")
                nc.tensor.matmul(vp[:], S[:], xcur[:], start=True, stop=True)
                us = spool.tile([64, 64], f32, tag="us")
                nc.vector.tensor_copy(us[:], up[0:64, 64:128])
                pm = spool.tile([64, 64], f32, tag="pm")
                nc.vector.tensor_tensor(
                    pm[:], us[:], vp[0:64, 64:128], mybir.AluOpType.mult
                )
                pcol = spool.tile([64, 1], f32, tag="pcol")
                nc.vector.reduce_sum(pcol[:], pm[:], axis=mybir.AxisListType.X)
                cp = epool.tile([1, 1], f32, tag="cp")
                nc.tensor.matmul(cp[:], pcol[:], onesf[0:64, 0:1], start=True, stop=True)
                c2 = spool.tile([1, 1], f32, tag="c2")
                nc.vector.tensor_copy(c2[:], cp[:])
                root = spool.tile([1, 1], f32, tag="root")
                nc.scalar.sqrt(root[:], c2[:])
                nc.vector.tensor_scalar_mul(root[:], root[:], -1.0)
                nc.sync.dma_start(out_d, root[:])

    nc.compile()
    return nc


def _get_nc():
    key = (GRAM_MODE, NEWTON_ITERS)
    if key not in _CACHE:
        _CACHE[key] = _build()
    return _CACHE[key]


def _const_inputs():
    eye = np.eye(128, dtype=np.float32)
    maskd = np.zeros((128, 128), dtype=np.float32)
    maskd[:64, :64] = np.eye(64, dtype=np.float32)
    maskd[64:, 64:] = np.eye(64, dtype=np.float32)
    return {
        "ident": np.eye(128).astype(_gnp()),
        "eye2": (2.0 * eye).astype(np.float32),
        "reye": (R_RIDGE * eye).astype(np.float32),
        "maskd": maskd,
        "onesf": np.ones((128, 128), dtype=np.float32),
    }


def kernel(data_view1, data_view2):
    from concourse import bass_utils

    h1 = np.ascontiguousarray(data_view1, dtype=np.float32).reshape(M, N, K)
    h2 = np.ascontiguousarray(data_view2, dtype=np.float32).reshape(M, N, K)

    consts = _const_inputs()
    in_maps = []
    for c in range(NC):
        m = {
            "x1": h1[c * NS : (c + 1) * NS],
            "x2": h2[c * NS : (c + 1) * NS],
        }
        m.update(consts)
        in_maps.append(m)

    nc = _get_nc()
    trace = os.environ.get("BASS_KERNEL_TRACE", "0") == "1"
    res = bass_utils.run_bass_kernel_spmd(
        nc, in_maps, core_ids=list(range(NC)), trace=trace
    )
    if trace:
        kernel.last_results = res
    val = np.asarray(res.results[0]["out"]).reshape(())
    return val.astype(np.float32)


# revision 7
# speedup vs baseline: 1.5477x; 1.1415x over previous
"""DCCA 2D loss kernel for 8 Trainium2 NeuronCores (Bass/Tile).

Strategy (data-parallel over the m = B*C = 2048 sample axis):
  - Each core gets 256 samples of both views. Per 4-sample "quad" it loads
    both views with 128-partition DMAs (partition = (sample-in-quad)*32 +
    n//2, so every DMA descriptor reads a contiguous 1KB row-pair), PE-
    transposes the four [128,128] slices, assembles per-sample fused tiles
    T_m = [H1_m^T | H2_m^T] (the n-axis lands in a fixed even/odd
    permutation, under which the final scalar is exactly invariant), and
    accumulates the fused Gram  G += T_m^T T_m  in PSUM.  G's 64x64 blocks
    are [[S11raw, S12raw], [S21raw, S22raw]].
  - AllGather of the per-core partial G (64KB), summed on every core.
  - Replicated epilogue without eigh:  with  S = c1*G + R*I  and
    A = blockdiag(S11, S22),   corr^2 = trace(S11^-1 S12 S22^-1 S12^T)
    = sum( (A^-1 S)[0:64,64:128] * (S A^-1)[0:64,64:128] ),
    where A^-1 comes from Newton-Schulz iterations (X <- X(2I - AX),
    X0 = (n/tr A) I; condition number ~1.02 so 4 iterations are exact to
    fp32).  Output = -sqrt(corr^2).
"""

import os

import numpy as np

# ---------------------------------------------------------------- constants
B, C, N, K = 32, 64, 64, 128
M = B * C                    # 2048 samples
NC = 8                       # cores
NS = M // NC                 # 256 samples per core
# Chunk schedule (samples per DMA chunk, per view). The first chunk is small
# so the PE can start within a few us instead of waiting for a full 2MB
# SWDGE descriptor generation.
CHUNKS = (16, 48, 64, 64, 64)
assert sum(CHUNKS) == NS
G = 8                        # samples per partition-block (2KB DMA runs)
R_RIDGE = 1e-4
C1 = float((1.0 - 1.0 / M) ** 2 / (M * (M - 1)))  # Gram -> Sigma scale

# "bf16" (fast, ~1e-4 rel err), "f32r" (reduced fp32 matmul), "f32" (safest)
GRAM_MODE = os.environ.get("BASS_GRAM_MODE", "bf16")
NEWTON_ITERS = int(os.environ.get("BASS_NEWTON_ITERS", "4"))
PIPELINE = os.environ.get("BASS_PIPELINE", "1") == "1"

_CACHE = {}


def _gdt(mybir):
    return {
        "bf16": mybir.dt.bfloat16,
        "f32r": mybir.dt.float32r,
        "f32": mybir.dt.float32,
    }[GRAM_MODE]


def _gnp():
    if GRAM_MODE == "bf16":
        import ml_dtypes

        return ml_dtypes.bfloat16
    return np.float32


def _build():
    import concourse.bass as bass
    import concourse.mybir as mybir
    import concourse.tile as tile
    from concourse import bacc

    gdt = _gdt(mybir)
    f32 = mybir.dt.float32

    nc = bacc.Bacc(
        "TRN2",
        target_bir_lowering=False,
        debug=False,
        enable_asserts=False,
        num_devices=NC,
    )

    x1 = nc.dram_tensor("x1", [NS, N, K], f32, kind="ExternalInput").ap()
    x2 = nc.dram_tensor("x2", [NS, N, K], f32, kind="ExternalInput").ap()
    ident_d = nc.dram_tensor("ident", [128, 128], gdt, kind="ExternalInput").ap()
    eye2_d = nc.dram_tensor("eye2", [128, 128], f32, kind="ExternalInput").ap()
    reye_d = nc.dram_tensor("reye", [128, 128], f32, kind="ExternalInput").ap()
    maskd_d = nc.dram_tensor("maskd", [128, 128], f32, kind="ExternalInput").ap()
    ones_d = nc.dram_tensor("onesf", [128, 128], f32, kind="ExternalInput").ap()
    out_d = nc.dram_tensor("out", [1, 1], f32, kind="ExternalOutput").ap()

    with tile.TileContext(nc) as tc:
        import contextlib

        with contextlib.ExitStack() as ctx:
            cpool = ctx.enter_context(tc.tile_pool(name="consts", bufs=1))
            spool = ctx.enter_context(tc.tile_pool(name="work", bufs=2))

            ident = cpool.tile([128, 128], gdt)
            nc.sync.dma_start(ident[:], ident_d)
            eye2 = cpool.tile([128, 128], f32)
            nc.sync.dma_start(eye2[:], eye2_d)
            reye = cpool.tile([128, 128], f32)
            nc.sync.dma_start(reye[:], reye_d)
            maskd = cpool.tile([128, 128], f32)
            nc.sync.dma_start(maskd[:], maskd_d)
            onesf = cpool.tile([128, 128], f32)
            nc.sync.dma_start(onesf[:], ones_d)

            gsb = spool.tile([128, 128], f32, tag="gsb")

            # ---------------- main loop: per-core partial fused Gram ------
            # Layout per chunk/view SBUF tile V [128, CH/G, G//2 * 128]:
            #   V[16h+u, j, r*128+k] = X[s0+G*j+h, 4u+r, k]
            # so each DMA descriptor reads (G/2) consecutive n-rows = 2KB.
            # Per G-sample block: 8 transposes (2 views x 4 r) emitted as
            # regular matmuls against the identity (keeps PE-HAM warm and
            # enables FWL for bf16, unlike transpose-mode), one assembly
            # copy per view into the fused TT tile, then G Gram matmuls.
            with (
                tc.tile_pool(name="vload", bufs=2) as vpool,
                tc.tile_pool(name="ttp", bufs=3) as ttpool,
                tc.tile_pool(name="ptp", bufs=2, space="PSUM") as ptpool,
                tc.tile_pool(name="gpp", bufs=1, space="PSUM") as gpool,
            ):
                if GRAM_MODE == "f32r":
                    gpA = gpool.tile([128, 256], f32, tag="gpA")
                    gpB = gpool.tile([128, 256], f32, tag="gpB")
                else:
                    gp = gpool.tile([128, 128], f32, tag="gp")

                first = [True, True]  # start-flags for (gpA, gpB) / (gp,)
                n_blocks_total = NS // G

                def emit_gram(tt8, bi):
                    last = bi == n_blocks_total - 1
                    tt8f = tt8.rearrange("p h b u -> p (h b u)")
                    if GRAM_MODE == "f32r":
                        for h in range(G):
                            acc = gpA if h % 2 == 0 else gpB
                            fi = h % 2
                            nc.tensor.matmul(
                                acc[:],
                                tt8f[:, 128 * h : 128 * (h + 1)],
                                tt8f[:, 256 * (h // 2) : 256 * (h // 2 + 1)],
                                start=first[fi],
                                stop=last and h >= G - 2,
                            )
                            first[fi] = False
                    else:
                        for h in range(G):
                            nc.tensor.matmul(
                                gp[:],
                                tt8f[:, 128 * h : 128 * (h + 1)],
                                tt8f[:, 128 * h : 128 * (h + 1)],
                                start=first[0],
                                stop=last and h == G - 1,
                            )
                            first[0] = False

                pending = None  # (tt8, block_index) awaiting Gram matmuls
                bi = 0
                s0 = 0
                for ci, CH in enumerate(CHUNKS):
                    nj = CH // G
                    vts = []
                    for vi, xsrc in enumerate((x1, x2)):
                        vt = vpool.tile([128, nj, 512], gdt, tag=f"v{vi}_{CH}")
                        src = xsrc[s0 : s0 + CH].rearrange(
                            "(j h) (u r) k -> (h u) j (r k)", h=G, r=4
                        )
                        if gdt == f32:
                            nc.sync.dma_start(vt[:], src)
                        else:
                            # SWDGE casts f32 -> bf16 (or relabels f32r)
                            # during the transfer.
                            nc.gpsimd.dma_start(vt[:], src)
                        vts.append(vt)
                    s0 += CH

                    for j in range(nj):
                        tt8 = ttpool.tile([128, G, 2, 64], gdt, tag="tt8")
                        for vi in range(2):
                            ptdt = f32 if GRAM_MODE == "bf16" else gdt
                            pt4 = ptpool.tile([128, 4, 128], ptdt, tag=f"pt{vi}")
                            for r in range(4):
                                if GRAM_MODE == "bf16":
                                    nc.tensor.matmul(
                                        pt4[:, r, :],
                                        vts[vi][:, j, r * 128 : (r + 1) * 128],
                                        ident[:],
                                        start=True,
                                        stop=True,
                                    )
                                else:
                                    nc.tensor.transpose(
                                        pt4[:, r, :],
                                        vts[vi][:, j, r * 128 : (r + 1) * 128],
                                        ident[:],
                                    )
                            nc.any.tensor_copy(
                                out=tt8[:, :, vi, :].rearrange(
                                    "p h (r u) -> p h r u", r=4
                                ),
                                in_=pt4.rearrange(
                                    "p r (h u) -> p h r u", h=G
                                ),
                            )
                        # one-block software pipeline: this block's Gram
                        # matmuls are emitted after the NEXT block's
                        # transposes, so the PE never stalls on the copy.
                        if PIPELINE:
                            if pending is not None:
                                emit_gram(*pending)
                            pending = (tt8, bi)
                        else:
                            emit_gram(tt8, bi)
                        bi += 1
                if pending is not None:
                    emit_gram(*pending)

                if GRAM_MODE == "f32r":
                    nc.vector.tensor_add(gsb[:], gpA[:, 0:128], gpB[:, 128:256])
                else:
                    nc.vector.tensor_copy(gsb[:], gp[:])

            # ---------------- AllGather + replicated epilogue -------------
            with (
                tc.tile_pool(name="dram", bufs=1, space="DRAM") as dpool,
                tc.tile_pool(name="epp", bufs=1, space="PSUM") as epool,
            ):
                din = dpool.tile([128, 128], f32)
                dout = dpool.tile([NC, 128, 128], f32)
                nc.gpsimd.dma_start(din[:], gsb[:])
                nc.gpsimd.collective_compute(
                    "AllGather",
                    mybir.AluOpType.bypass,
                    replica_groups=[list(range(NC))],
                    ins=[din.opt()],
                    outs=[dout.opt()],
                )
                gall = spool.tile([128, NC, 128], f32, tag="gall")
                nc.sync.dma_start(gall[:], dout[:].rearrange("c p k -> p c k"))

                g4 = spool.tile([128, 4, 128], f32, tag="g4")
                nc.vector.tensor_add(g4[:], gall[:, 0:4, :], gall[:, 4:8, :])
                g2 = spool.tile([128, 2, 128], f32, tag="g2")
                nc.vector.tensor_add(g2[:], g4[:, 0:2, :], g4[:, 2:4, :])
                gs = spool.tile([128, 128], f32, tag="gs")
                nc.vector.tensor_add(gs[:], g2[:, 0, :], g2[:, 1, :])

                # S = c1*G + R*I ; A = blockdiag(S)
                S = spool.tile([128, 128], f32, tag="S")
                nc.vector.tensor_scalar_mul(S[:], gs[:], C1)
                nc.vector.tensor_add(S[:], S[:], reye[:])
                A = spool.tile([128, 128], f32, tag="A")
                nc.vector.tensor_mul(A[:], S[:], maskd[:])

                # X0 = (128 / tr(A)) * I   via (A*2I) row-sums -> ones bcast
                dm = spool.tile([128, 128], f32, tag="dm")
                nc.vector.tensor_mul(dm[:], A[:], eye2[:])
                dcol = spool.tile([128, 1], f32, tag="dcol")
                nc.vector.reduce_sum(dcol[:], dm[:], axis=mybir.AxisListType.X)
                trp = epool.tile([128, 1], f32, tag="trp")
                nc.tensor.matmul(trp[:], onesf[:], dcol[:], start=True, stop=True)
                trs = spool.tile([128, 1], f32, tag="trs")
                nc.vector.tensor_copy(trs[:], trp[:])
                rcol = spool.tile([128, 1], f32, tag="rcol")
                nc.vector.reciprocal(rcol[:], trs[:])
                xcur = spool.tile([128, 128], f32, tag="xn")
                nc.vector.tensor_scalar(
                    xcur[:], eye2[:], rcol[:], 128.0,
                    op0=mybir.AluOpType.mult, op1=mybir.AluOpType.mult,
                )

                for _ in range(NEWTON_ITERS):
                    bp = epool.tile([128, 128], f32, tag="bp")
                    nc.tensor.matmul(bp[:], A[:], xcur[:], start=True, stop=True)
                    cs = spool.tile([128, 128], f32, tag="cs")
                    nc.vector.tensor_tensor(
                        cs[:], eye2[:], bp[:], mybir.AluOpType.subtract
                    )
                    xp = epool.tile([128, 128], f32, tag="xp")
                    nc.tensor.matmul(xp[:], xcur[:], cs[:], start=True, stop=True)
                    xnew = spool.tile([128, 128], f32, tag="xn")
                    nc.vector.tensor_copy(xnew[:], xp[:])
                    xcur = xnew

                up = epool.tile([128, 128], f32, tag="up")
                nc.tensor.matmul(up[:], xcur[:], S[:], start=True, stop=True)
                vp = epool.tile([128, 128], f32, tag="vp")
                nc.tensor.matmul(vp[:], S[:], xcur[:], start=True, stop=True)
                us = spool.tile([64, 64], f32, tag="us")
                nc.vector.tensor_copy(us[:], up[0:64, 64:128])
                pm = spool.tile([64, 64], f32, tag="pm")
                nc.vector.tensor_tensor(
                    pm[:], us[:], vp[0:64, 64:128], mybir.AluOpType.mult
                )
                pcol = spool.tile([64, 1], f32, tag="pcol")
                nc.vector.reduce_sum(pcol[:], pm[:], axis=mybir.AxisListType.X)
                cp = epool.tile([1, 1], f32, tag="cp")
                nc.tensor.matmul(cp[:], pcol[:], onesf[0:64, 0:1], start=True, stop=True)
                c2 = spool.tile([1, 1], f32, tag="c2")
                nc.vector.tensor_copy(c2[:], cp[:])
                root = spool.tile([1, 1], f32, tag="root")
                nc.scalar.sqrt(root[:], c2[:])
                nc.vector.tensor_scalar_mul(root[:], root[:], -1.0)
                nc.sync.dma_start(out_d, root[:])

    nc.compile()
    return nc


def _get_nc():
    key = (GRAM_MODE, NEWTON_ITERS)
    if key not in _CACHE:
        _CACHE[key] = _build()
    return _CACHE[key]


def _const_inputs():
    eye = np.eye(128, dtype=np.float32)
    maskd = np.zeros((128, 128), dtype=np.float32)
    maskd[:64, :64] = np.eye(64, dtype=np.float32)
    maskd[64:, 64:] = np.eye(64, dtype=np.float32)
    return {
        "ident": np.eye(128).astype(_gnp()),
        "eye2": (2.0 * eye).astype(np.float32),
        "reye": (R_RIDGE * eye).astype(np.float32),
        "maskd": maskd,
        "onesf": np.ones((128, 128), dtype=np.float32),
    }


def kernel(data_view1, data_view2):
    from concourse import bass_utils

    h1 = np.ascontiguousarray(data_view1, dtype=np.float32).reshape(M, N, K)
    h2 = np.ascontiguousarray(data_view2, dtype=np.float32).reshape(M, N, K)

    consts = _const_inputs()
    in_maps = []
    for c in range(NC):
        m = {
            "x1": h1[c * NS : (c + 1) * NS],
            "x2": h2[c * NS : (c + 1) * NS],
        }
        m.update(consts)
        in_maps.append(m)

    nc = _get_nc()
    trace = os.environ.get("BASS_KERNEL_TRACE", "0") == "1"
    res = bass_utils.run_bass_kernel_spmd(
        nc, in_maps, core_ids=list(range(NC)), trace=trace
    )
    if trace:
        kernel.last_results = res
    val = np.asarray(res.results[0]["out"]).reshape(())
    return val.astype(np.float32)
